# revision 1
# baseline (speedup 1.0000x reference)
import os
import sys

sys.path.insert(0, "/opt/trn_rl_repo")

import numpy as np
import bass_rust
from concourse import bass, mybir
from concourse.tile import TileContext
from concourse.vector_clock import ScopedClock
from concourse.bass_utils import run_bass_kernel_spmd

B, S, E, H = 4, 2048, 1024, 1024
NCORES = 8
NT = 8  # q-tiles per core, 128 rows each
F32 = mybir.dt.float32
F32R = mybir.dt.float32r

# Results of the last run_bass_kernel_spmd call (for test harness inspection).
LAST_RESULT = None


def _global_tile(core: int, t: int) -> int:
    """Local q-tile t on core -> global 128-row tile index r in [0, 16).

    Tiles are grouped in four causal classes (nb = t//2 + 1 key-blocks of
    512); each core takes two tiles per class so instruction streams are
    identical across cores.
    """
    half = core % 2
    return 4 * (t // 2) + 2 * half + (t % 2)


class PatchedTileContext(TileContext):
    """TileContext whose tail drain carries at most one sem wait.

    The walrus codegen in this container rejects a Drain with more than one
    sync wait ("Too many sync wait commands"); split the global-clock waits
    across a chain of drains on the same engine instead.
    """

    def _drain_and_barrier(self, tick_clock, wait_clock):
        drain_inst = self.nc.sync.drain()
        wait_clock.add_sem_waits(
            drain_inst.ins, ScopedClock({None: tick_clock.global_clock})
        )
        mi = drain_inst.ins
        waits = list(mi.sync_info.on_wait)
        ups = list(mi.sync_info.on_update)
        if len(waits) > 1:
            mi.sync_info = bass_rust.SyncInfo(on_wait=waits[:1], on_update=[])
            for i, w in enumerate(waits[1:]):
                d2 = self.nc.sync.drain()
                last = i == len(waits) - 2
                d2.ins.sync_info = bass_rust.SyncInfo(
                    on_wait=[w], on_update=ups if last else []
                )
        self.nc.all_engine_barrier()
        assert self.sems is not None
        popped = self.nc._tile_sem_poison_stack.pop()
        assert popped is self._sem_poison
        self.nc.clear_and_free_semaphores(list(self.sems.allocated().values()))
        self.nc.all_engine_barrier()


def _split_multi_waits(json_bytes):
    """Rewrite BIR so no instruction carries more than one sync wait.

    The walrus build in this container rejects instructions with multiple
    sync waits ("Too many sync wait commands"). Engines execute in order, so
    hoisting the extra waits onto NoOp instructions inserted immediately
    before the original instruction is semantically equivalent.
    """
    import json as _json

    d = _json.loads(json_bytes)
    ctr = 0
    for f in d.get("functions", []):
        for blk in f.get("blocks", []):
            insts = blk.get("instructions", [])
            out = []
            for inst in insts:
                si = inst.get("sync_info") or {}
                ow = si.get("on_wait") or []
                if len(ow) > 1:
                    for w in ow[:-1]:
                        out.append(
                            {
                                "debug": inst.get("debug", 0),
                                "engine": inst["engine"],
                                "ins": [],
                                "name": f"wsplit_{ctr}",
                                "opcode": "NoOp",
                                "outs": [],
                                "sync_info": {"on_update": [], "on_wait": [w]},
                            }
                        )
                        ctr += 1
                    si = dict(si)
                    si["on_wait"] = [ow[-1]]
                    inst = dict(inst)
                    inst["sync_info"] = si
                out.append(inst)
            blk["instructions"] = out
    return _json.dumps(d).encode()


def _build_program():
    nc = bass.Bass("TRN2", target_bir_lowering=False, debug=False, num_devices=NCORES)
    orig_to_json_bytes = nc.to_json_bytes
    nc.to_json_bytes = lambda: _split_multi_waits(orig_to_json_bytes())

    xT = nc.dram_tensor("xT", [E, S], F32R, kind="ExternalInput")
    xq = nc.dram_tensor("xq", [E, NT * 128], F32R, kind="ExternalInput")
    wqT = nc.dram_tensor("wqT", [E, H], F32R, kind="ExternalInput")
    wkT = nc.dram_tensor("wkT", [E, H], F32R, kind="ExternalInput")
    wvT = nc.dram_tensor("wvT", [E, H], F32R, kind="ExternalInput")
    bqs = nc.dram_tensor("bqs", [H], F32, kind="ExternalInput")
    bk = nc.dram_tensor("bk", [H], F32, kind="ExternalInput")
    bv = nc.dram_tensor("bv", [H], F32, kind="ExternalInput")
    masks = nc.dram_tensor("masks", [NT, 128, 512], F32, kind="ExternalInput")
    ident = nc.dram_tensor("ident", [128, 128], F32, kind="ExternalInput")
    out = nc.dram_tensor("out", [NT, 128, H], F32, kind="ExternalOutput")

    EC = E // 128  # 8 contraction chunks
    HC = H // 128  # 8 h-chunks

    with PatchedTileContext(nc) as tc:
        with (
            tc.tile_pool(name="const", bufs=1) as const_pool,
            tc.tile_pool(name="vlo", bufs=1) as vlo_pool,
            tc.tile_pool(name="kt", bufs=1) as kt_pool,
            tc.tile_pool(name="stream", bufs=2) as stream_pool,
            tc.tile_pool(name="dram", bufs=1, space="DRAM") as dram_pool,
        ):
            cst = const_pool.tile([128, H + 2 * HC], F32, tag="cst")
            bv_bc = cst[:, 0:H]
            bq_t = cst[:, H : H + HC]
            bk_t = cst[:, H + HC : H + 2 * HC]
            idc = const_pool.tile([128, 128], F32, tag="idc")
            id_t = idc[:, :]
            nc.gpsimd.dma_start(out=bq_t, in_=bqs[:].rearrange("(c p) -> p c", p=128))
            nc.gpsimd.dma_start(out=bk_t, in_=bk[:].rearrange("(c p) -> p c", p=128))
            nc.gpsimd.dma_start(out=bv_bc, in_=bv[:].partition_broadcast(128))
            nc.sync.dma_start(out=id_t, in_=ident[:, :])

            v_lo = [
                vlo_pool.tile([128, H], F32R, tag=f"vlo{i}", name=f"vlo{i}")
                for i in range(8)
            ]
            kt = [
                kt_pool.tile([128, S], F32R, tag=f"kt{c}", name=f"kt{c}")
                for c in range(HC)
            ]
            v_hi = dram_pool.tile([8, 128, H], F32R, tag="vhi")

            # Weight pools: slot-reuse via shared tags (wk -> wq, wv -> xq)
            # gives prefetch overlap without violating pool stack order.
            with (
                tc.tile_pool(name="wA", bufs=1) as wA_pool,
                tc.tile_pool(name="wB", bufs=1) as wB_pool,
            ):
                # ---- phase 0a: K projection (kT-first: PE starts early) --
                with tc.tile_pool(name="kvps", bufs=8, space="PSUM") as kvps_pool:
                    wk_sb = [
                        wA_pool.tile([128, H], F32R, tag=f"wA{e}", name=f"wk{e}")
                        for e in range(EC)
                    ]
                    wv_sb = [
                        wB_pool.tile([128, H], F32R, tag=f"wB{e}", name=f"wv{e}")
                        for e in range(EC)
                    ]
                    for sl in range(4):  # 512-wide key slices
                        xsl = stream_pool.tile([128, EC, 512], F32R, tag="xsl")
                        if sl == 0:
                            # interleave x chunks with wk chunks in e order so
                            # the e=0 matmul can issue after ~0.75MB of DMA,
                            # then prefetch wv behind them
                            for e in range(EC):
                                nc.sync.dma_start(
                                    out=xsl[:, e, :],
                                    in_=xT[e * 128 : (e + 1) * 128, 0:512],
                                )
                                nc.sync.dma_start(
                                    out=wk_sb[e], in_=wkT[e * 128 : (e + 1) * 128, :]
                                )
                        else:
                            for e in range(EC):
                                nc.sync.dma_start(
                                    out=xsl[:, e, :],
                                    in_=xT[
                                        e * 128 : (e + 1) * 128,
                                        sl * 512 : (sl + 1) * 512,
                                    ],
                                )
                            if sl == 1:
                                for e in range(EC):
                                    nc.sync.dma_start(
                                        out=wv_sb[e],
                                        in_=wvT[e * 128 : (e + 1) * 128, :],
                                    )
                        psk = [
                            kvps_pool.tile([128, 512], F32, tag="kvps", name=f"psk{hc}")
                            for hc in range(HC)
                        ]
                        for e in range(EC):
                            for hc in range(HC):
                                nc.tensor.matmul(
                                    psk[hc],
                                    lhsT=wk_sb[e][:, hc * 128 : (hc + 1) * 128],
                                    rhs=xsl[:, e, :],
                                    start=(e == 0),
                                    stop=(e == EC - 1),
                                )
                        for hc in range(HC):
                            nc.vector.tensor_scalar_add(
                                kt[hc][:, sl * 512 : (sl + 1) * 512],
                                psk[hc],
                                bk_t[:, hc : hc + 1],
                            )

                    # ---- phase 0b: V projection (second pass over xT) ----
                    # wq prefetches into the wk slots (tag reuse -> anti-dep)
                    wq_sb = [
                        wA_pool.tile([128, H], F32R, tag=f"wA{e}", name=f"wq{e}")
                        for e in range(EC)
                    ]
                    for e in range(EC):
                        nc.sync.dma_start(
                            out=wq_sb[e], in_=wqT[e * 128 : (e + 1) * 128, :]
                        )
                    with tc.tile_pool(name="vbnc", bufs=2) as vbnc_pool:
                        for sl in range(4):
                            xsl = stream_pool.tile([128, EC, 512], F32R, tag="xsl")
                            for e in range(EC):
                                nc.sync.dma_start(
                                    out=xsl[:, e, :],
                                    in_=xT[
                                        e * 128 : (e + 1) * 128,
                                        sl * 512 : (sl + 1) * 512,
                                    ],
                                )
                            psv = [
                                [
                                    kvps_pool.tile(
                                        [128, 512],
                                        F32,
                                        tag="kvps",
                                        name=f"psv{si}_{hh}",
                                    )
                                    for hh in range(2)
                                ]
                                for si in range(4)
                            ]
                            for e in range(EC):
                                for si in range(4):
                                    for hh in range(2):
                                        nc.tensor.matmul(
                                            psv[si][hh],
                                            lhsT=xsl[:, e, si * 128 : (si + 1) * 128],
                                            rhs=wv_sb[e][:, hh * 512 : (hh + 1) * 512],
                                            start=(e == 0),
                                            stop=(e == EC - 1),
                                        )
                            for si in range(4):
                                kc = sl * 4 + si
                                for hh in range(2):
                                    hs = slice(hh * 512, (hh + 1) * 512)
                                    if kc < 8:
                                        nc.vector.tensor_add(
                                            v_lo[kc][:, hs], psv[si][hh], bv_bc[:, hs]
                                        )
                                    else:
                                        bnc = vbnc_pool.tile(
                                            [128, 512], F32R, tag="vbnc"
                                        )
                                        nc.vector.tensor_add(
                                            bnc, psv[si][hh], bv_bc[:, hs]
                                        )
                                        nc.sync.dma_start(
                                            out=v_hi[kc - 8, :, hs], in_=bnc
                                        )

                # ---- phase 1: Q projection ------------------------------
                # xq reuses the wv slots; qT lands in the stream slots and
                # stays resident through attention.
                with tc.tile_pool(name="qps", bufs=8, space="PSUM") as qps_pool:
                    xq_sb = [
                        wB_pool.tile([128, NT * 128], F32R, tag=f"wB{e}", name=f"xq{e}")
                        for e in range(EC)
                    ]
                    for qh in range(2):
                        qcols = slice(qh * 512, (qh + 1) * 512)
                        for e in range(EC):
                            nc.sync.dma_start(
                                out=xq_sb[e][:, qcols],
                                in_=xq[e * 128 : (e + 1) * 128, qcols],
                            )
                    qt_grp = [
                        stream_pool.tile(
                            [128, 4, HC, 128], F32R, tag="xsl", name=f"qtg{g}"
                        )
                        for g in range(2)
                    ]
                    qt_sb = [qt_grp[t // 4][:, t % 4, :, :] for t in range(NT)]
                    for qs in range(2):  # 512-wide query column groups
                        ps = [
                            qps_pool.tile([128, 512], F32, tag="qps", name=f"qps{hc}")
                            for hc in range(HC)
                        ]
                        for e in range(EC):
                            for hc in range(HC):
                                nc.tensor.matmul(
                                    ps[hc],
                                    lhsT=wq_sb[e][:, hc * 128 : (hc + 1) * 128],
                                    rhs=xq_sb[e][:, qs * 512 : (qs + 1) * 512],
                                    start=(e == 0),
                                    stop=(e == EC - 1),
                                )
                        g = qs  # query group g holds tiles qs*4..qs*4+3
                        for hc in range(HC):
                            nc.vector.tensor_scalar_add(
                                qt_grp[g][:, :, hc, :],
                                ps[hc][:, :].rearrange("p (j q) -> p j q", j=4),
                                bq_t[:, hc : hc + 1],
                            )

            # ---- phase 2: attention -------------------------------------
            with (
                tc.tile_pool(name="mskp", bufs=2) as msk_pool,
                tc.tile_pool(name="ssb", bufs=2) as ssb_pool,
                tc.tile_pool(name="ptp", bufs=4) as pt_pool,
                tc.tile_pool(name="vhip", bufs=2) as vhi_pool,
                tc.tile_pool(name="outp", bufs=2) as out_pool,
                tc.tile_pool(name="stat", bufs=8) as stat_pool,
                tc.tile_pool(name="sps", bufs=4, space="PSUM") as sps_pool,
                tc.tile_pool(name="ops", bufs=2, space="PSUM") as ops_pool,
                tc.tile_pool(name="tps", bufs=2, space="PSUM") as tps_pool,
            ):
                for cls in range(4):
                    nb = cls + 1
                    nkc = nb * 4
                    # v chunks >= 8 for this class, loaded once for both
                    # tiles, packed 4 chunks per slot
                    ngrp = max(0, (nkc - 8 + 3) // 4)
                    vh_grp = []
                    for g in range(ngrp):
                        vh = vhi_pool.tile(
                            [128, 4, H], F32R, tag="vhg", name=f"vhg{cls}_{g}"
                        )
                        for j in range(4):
                            nc.sync.dma_start(
                                out=vh[:, j, :], in_=v_hi[g * 4 + j, :, :]
                            )
                        vh_grp.append(vh)
                    for t in (2 * cls, 2 * cls + 1):
                        qt = qt_sb[t]
                        msk = msk_pool.tile([128, 512], F32, tag="msk")
                        nc.sync.dma_start(out=msk, in_=masks[t, :, :])

                        ssb = ssb_pool.tile([128, 4, 512], F32, tag="ssb")
                        mparts = stat_pool.tile([128, 4], F32, tag="mparts")
                        for kb in range(nb):
                            sp = sps_pool.tile([128, 512], F32, tag="sp")
                            for hc in range(HC):
                                nc.tensor.matmul(
                                    sp,
                                    lhsT=qt[:, hc, :],
                                    rhs=kt[hc][:, kb * 512 : (kb + 1) * 512],
                                    start=(hc == 0),
                                    stop=(hc == HC - 1),
                                )
                            if kb == nb - 1:
                                nc.vector.tensor_add(ssb[:, kb, :], sp, msk)
                            else:
                                nc.vector.tensor_copy(ssb[:, kb, :], sp)
                            # per-block partial max overlaps later score blocks
                            nc.vector.reduce_max(
                                mparts[:, kb : kb + 1],
                                ssb[:, kb, :],
                                axis=mybir.AxisListType.X,
                            )
                        nm = stat_pool.tile([128, 1], F32, tag="nm")
                        nc.vector.reduce_max(
                            nm, mparts[:, :nb], axis=mybir.AxisListType.X, negate=True
                        )
                        # per-block exp lets the first transposes start while
                        # later blocks are still exponentiating
                        lparts = stat_pool.tile([128, 4], F32, tag="lparts")
                        for kb in range(nb):
                            nc.scalar.activation(
                                ssb[:, kb, :],
                                ssb[:, kb, :],
                                mybir.ActivationFunctionType.Exp,
                                bias=nm,
                                accum_out=lparts[:, kb : kb + 1],
                            )
                        lsum = stat_pool.tile([128, 1], F32, tag="lsum")
                        nc.vector.reduce_sum(
                            lsum, lparts[:, :nb], axis=mybir.AxisListType.X
                        )
                        rl = stat_pool.tile([128, 1], F32, tag="rl")
                        nc.vector.reciprocal(rl, lsum)

                        po = [
                            ops_pool.tile([128, 512], F32, tag="po", name=f"po{hh}")
                            for hh in range(2)
                        ]
                        for kc in range(nkc):
                            tp = tps_pool.tile([128, 128], F32, tag="tp")
                            nc.tensor.transpose(
                                tp,
                                ssb[:, kc // 4, (kc % 4) * 128 : (kc % 4 + 1) * 128],
                                id_t,
                            )
                            pt = pt_pool.tile([128, 128], F32R, tag="pt")
                            nc.vector.tensor_copy(pt, tp)
                            if kc < 8:
                                vk = v_lo[kc]
                            else:
                                vk = vh_grp[(kc - 8) // 4][:, (kc - 8) % 4, :]
                            for hh in range(2):
                                nc.tensor.matmul(
                                    po[hh],
                                    lhsT=pt,
                                    rhs=vk[:, hh * 512 : (hh + 1) * 512],
                                    start=(kc == 0),
                                    stop=(kc == nkc - 1),
                                )

                        ot = out_pool.tile([128, H], F32, tag="ot")
                        for hh in range(2):
                            nc.vector.tensor_scalar_mul(
                                ot[:, hh * 512 : (hh + 1) * 512], po[hh], rl
                            )
                        nc.sync.dma_start(out=out[t, :, :], in_=ot)

    return nc


def kernel(inputs, Wq, bq, Wk, bk, Wv, bv):
    global LAST_RESULT
    inputs = np.ascontiguousarray(inputs, dtype=np.float32)
    scale = 1.0 / np.sqrt(np.float32(E))

    wqT = np.ascontiguousarray(Wq.T.astype(np.float32) * scale)
    wkT = np.ascontiguousarray(Wk.T.astype(np.float32))
    wvT = np.ascontiguousarray(Wv.T.astype(np.float32))
    bqs = (bq.astype(np.float32) * scale).copy()
    bk = np.ascontiguousarray(bk, dtype=np.float32)
    bv = np.ascontiguousarray(bv, dtype=np.float32)
    ident = np.eye(128, dtype=np.float32)

    xTs = [np.ascontiguousarray(inputs[b].T) for b in range(B)]

    in_maps = []
    for c in range(NCORES):
        b = c // 2
        xT = xTs[b]
        cols = []
        mask = np.empty((NT, 128, 512), dtype=np.float32)
        for t in range(NT):
            r = _global_tile(c, t)
            nb = t // 2 + 1
            cols.append(xT[:, r * 128 : (r + 1) * 128])
            jg = (nb - 1) * 512 + np.arange(512)[None, :]
            ig = r * 128 + np.arange(128)[:, None]
            mask[t] = np.where(jg <= ig, 0.0, -1e30).astype(np.float32)
        xq = np.ascontiguousarray(np.concatenate(cols, axis=1))
        in_maps.append(
            {
                "xT": xT,
                "xq": xq,
                "wqT": wqT,
                "wkT": wkT,
                "wvT": wvT,
                "bqs": bqs,
                "bk": bk,
                "bv": bv,
                "masks": mask,
                "ident": ident,
            }
        )

    nc = _build_program()
    res = None
    last_err = None
    for attempt in range(3):
        try:
            res = run_bass_kernel_spmd(nc, in_maps, list(range(NCORES)))
            break
        except Exception as e:  # transient NRT device wedge; retry
            last_err = e
            import time as _time

            _time.sleep(2.0)
    if res is None:
        raise last_err
    LAST_RESULT = res

    out = np.empty((B, S, H), dtype=np.float32)
    for c in range(NCORES):
        b = c // 2
        o = res.results[c]["out"]  # [NT, 128, H]
        for t in range(NT):
            r = _global_tile(c, t)
            out[b, r * 128 : (r + 1) * 128, :] = o[t]
    return out

